# revision 1
# baseline (speedup 1.0000x reference)
"""CommNet forward on 8 TRN2 NeuronCores (Bass/Tile).

Model (per reference):
    h0 = emb[agent_ids]                      # (B, M, H)
    repeat 4x:
        c = (sum_m h - h) / (M-1)
        x = [h, c, h0]                       # (B, M, 3H)
        d = relu(x @ W1 + b1) @ W2 + b2
        h = h + d
    logits = h @ Wd + bd                     # (B, M, A)

Constants: B=1024, M=64, H=256, A=16, V=1000, 4 comm steps.

Sharding: data-parallel on B across 8 cores (128 groups / core); weights
replicated. Within a core every tensor is laid out [hidden-on-partitions,
tokens-on-free] (tokens = group*64 + agent, T=8192 per core).

Algebra used on-device (host folds weights accordingly):
    x @ W1 = h @ (W1h - inv*W1c) + S @ (inv*W1c) + h0 @ W1h0
with S = sum_m h broadcast per group, inv = 1/(M-1).  z0 = h0 @ W1h0 + b1 is
precomputed once; per step the PSUM accumulation is:
    psum1 = W1hp.T@h(K=2) + I.T@z0b + I.T@bcast(SW)      -> d1 = relu(psum1)
    psum2 = W2.T@d1(K=2)  + I.T@h                        -> h  = psum2 + b2
Matmuls run as float32r (fp32 bits, tf32-class rounding, 1 cyc/row).
"""

import numpy as np

B, M, H, A, V = 1024, 64, 256, 16, 1000
STEPS = 4
NCORES = 8
G = B // NCORES          # groups per core = 128
T = G * M                # tokens per core = 8192
P = 128                  # partitions
KT = H // P              # K tiles per H = 2
NCH = T // 512           # 512-token chunks = 16
CH = 512
GPC = CH // M            # groups per chunk = 8
INV = 1.0 / (M - 1)

_CACHE = {}


def _build():
    import concourse.bass as bass
    import concourse.tile as tile
    from concourse import bacc, mybir
    from concourse.masks import make_identity

    F32 = mybir.dt.float32
    F32R = mybir.dt.float32r
    I32 = mybir.dt.int32

    nc = bacc.Bacc("TRN2", target_bir_lowering=False, debug=False,
                   num_devices=NCORES)

    ids_d = nc.dram_tensor("ids_pt", [P, T // P], I32, kind="ExternalInput").ap()
    emb_d = nc.dram_tensor("emb", [V, H], F32, kind="ExternalInput").ap()
    w1hp_d = nc.dram_tensor("w1hp", [P, KT, H], F32, kind="ExternalInput").ap()
    w1h0_d = nc.dram_tensor("w1h0", [P, KT, H], F32, kind="ExternalInput").ap()
    w1ci_d = nc.dram_tensor("w1ci", [P, KT, H], F32, kind="ExternalInput").ap()
    w2_d = nc.dram_tensor("w2", [P, KT, H], F32, kind="ExternalInput").ap()
    wd_d = nc.dram_tensor("wd", [P, KT, A], F32, kind="ExternalInput").ap()
    b1_d = nc.dram_tensor("b1p", [P, KT], F32, kind="ExternalInput").ap()
    bs_d = nc.dram_tensor("bsteps", [P, KT, STEPS], F32, kind="ExternalInput").ap()
    bd_d = nc.dram_tensor("bdp", [A, 1], F32, kind="ExternalInput").ap()
    logT_d = nc.dram_tensor("logT", [A, T], F32, kind="ExternalOutput").ap()

    with tile.TileContext(nc) as tc:
        with (
            tc.tile_pool(name="const", bufs=1) as const,
            tc.tile_pool(name="big", bufs=1) as big,
            tc.tile_pool(name="stage", bufs=2) as stage,
            tc.tile_pool(name="gat", bufs=4) as gat,
            tc.tile_pool(name="d1p", bufs=2) as d1p,
            tc.tile_pool(name="swp", bufs=2) as swp,
            tc.tile_pool(name="lgt", bufs=2) as lgt,
            tc.tile_pool(name="ps", bufs=2, space="PSUM") as ps,
        ):
            # ---- constants / weights -------------------------------------
            ids = const.tile([P, T // P], I32)
            nc.sync.dma_start(out=ids[:], in_=ids_d[:])

            ident = const.tile([P, P], F32)
            make_identity(nc, ident[:])
            ident_r = const.tile([P, P], F32R)
            nc.vector.tensor_copy(ident_r[:], ident[:])

            w1ci = const.tile([P, KT, H], F32)
            nc.sync.dma_start(out=w1ci[:], in_=w1ci_d[:])
            b1 = const.tile([P, KT], F32)
            nc.sync.dma_start(out=b1[:], in_=b1_d[:])
            bsteps = const.tile([P, KT, STEPS], F32)
            nc.sync.dma_start(out=bsteps[:], in_=bs_d[:])
            bd = const.tile([A, 1], F32)
            nc.sync.dma_start(out=bd[:], in_=bd_d[:])

            def load_r(dram, shape, name):
                st = stage.tile(shape, F32, tag="wstage", name=f"st_{name}")
                nc.sync.dma_start(out=st[:], in_=dram)
                t = const.tile(shape, F32R, tag=name, name=name)
                nc.vector.tensor_copy(t[:], st[:])
                return t

            w1hp = load_r(w1hp_d[:], [P, KT, H], "w1hp_r")
            w1h0 = load_r(w1h0_d[:], [P, KT, H], "w1h0_r")
            w2 = load_r(w2_d[:], [P, KT, H], "w2_r")
            wd = load_r(wd_d[:], [P, KT, A], "wd_r")

            # ---- big state tiles -----------------------------------------
            h = [big.tile([P, T], F32R, tag=f"h{k}", name=f"h{k}") for k in range(KT)]
            z0b = [big.tile([P, T], F32R, tag=f"z0b{k}", name=f"z0b{k}") for k in range(KT)]
            # S double-buffered across steps: S[par][k] [P, G] fp32
            S = [[big.tile([P, G], F32, tag=f"S{par}{k}", name=f"S{par}{k}") for k in range(KT)]
                 for par in range(2)]

            # ---- phase 1: gather h0 rows, transpose into h ---------------
            for t in range(T // P):
                gst = gat.tile([P, H], F32, tag="gst")
                nc.gpsimd.indirect_dma_start(
                    out=gst[:],
                    out_offset=None,
                    in_=emb_d[:],
                    in_offset=bass.IndirectOffsetOnAxis(ap=ids[:, t:t + 1], axis=0),
                )
                for k in range(KT):
                    pt = ps.tile([P, P], F32, space="PSUM", tag=f"mm1_{k}")
                    nc.tensor.transpose(out=pt[:], in_=gst[:, k * P:(k + 1) * P],
                                        identity=ident[:])
                    if (t + k) % 2 == 0:
                        nc.vector.tensor_copy(h[k][:, t * P:(t + 1) * P], pt[:])
                    else:
                        nc.scalar.activation(
                            out=h[k][:, t * P:(t + 1) * P], in_=pt[:],
                            func=mybir.ActivationFunctionType.Identity)

            # ---- phase 2: z0b = h0 @ W1h0 + b1 ; S0 = segsum(h0) ---------
            for q in range(NCH):
                qs = slice(q * CH, (q + 1) * CH)
                for j in range(KT):
                    pz = ps.tile([P, CH], F32, space="PSUM", tag=f"mm2_{j}")
                    for k in range(KT):
                        nc.tensor.matmul(
                            pz[:], w1h0[:, k, j * P:(j + 1) * P], h[k][:, qs],
                            start=(k == 0), stop=(k == KT - 1))
                    nc.scalar.activation(
                        out=z0b[j][:, qs], in_=pz[:],
                        func=mybir.ActivationFunctionType.Identity,
                        bias=b1[:, j:j + 1])
                gq = slice(q * GPC, (q + 1) * GPC)
                for k in range(KT):
                    nc.vector.tensor_reduce(
                        out=S[0][k][:, gq],
                        in_=h[k][:, qs].bitcast(F32).rearrange(
                            "p (g m) -> p g m", m=M),
                        axis=mybir.AxisListType.X, op=mybir.AluOpType.add)

            # ---- phase 3: comm steps -------------------------------------
            for s in range(STEPS):
                Scur, Snxt = S[s % 2], S[(s + 1) % 2]
                last = s == STEPS - 1
                HQ = NCH // 2  # chunks per group-half
                HLF = G // 2   # groups per half
                swsb = None
                for q in range(NCH):
                    if q % HQ == 0:
                        # SW for this half: (inv*W1c).T @ S[:, half]  [P, HLF]
                        # Computed per half so step s can start its first
                        # chunks before step s-1 finished its second half.
                        half = q // HQ
                        hs = slice(half * HLF, (half + 1) * HLF)
                        swsb = []
                        for j in range(KT):
                            psw = ps.tile([P, HLF], F32, space="PSUM",
                                          tag=f"mm1_{j}", name="psw")
                            for k in range(KT):
                                nc.tensor.matmul(
                                    psw[:], w1ci[:, k, j * P:(j + 1) * P],
                                    Scur[k][:, hs],
                                    start=(k == 0), stop=(k == KT - 1))
                            sw = swp.tile([P, HLF], F32R,
                                          tag=f"sw{half}{j}", name="sw")
                            nc.vector.tensor_copy(sw[:], psw[:])
                            swsb.append(sw)
                    qs = slice(q * CH, (q + 1) * CH)
                    gq = slice(q * GPC, (q + 1) * GPC)
                    lgq = slice((q % HQ) * GPC, (q % HQ + 1) * GPC)
                    d1 = []
                    for j in range(KT):
                        p1 = ps.tile([P, CH], F32, space="PSUM", tag=f"mm1_{j}")
                        for k in range(KT):
                            nc.tensor.matmul(
                                p1[:], w1hp[:, k, j * P:(j + 1) * P], h[k][:, qs],
                                start=(k == 0), stop=False)
                        nc.tensor.matmul(p1[:], ident_r[:], z0b[j][:, qs],
                                         start=False, stop=False)
                        nc.tensor.matmul(
                            p1[:].rearrange("p (g m) -> p g m", g=GPC),
                            ident_r[:],
                            swsb[j][:, lgq].to_broadcast([P, GPC, M]),
                            start=False, stop=True)
                        d = d1p.tile([P, CH], F32R, tag=f"d1_{j}")
                        nc.scalar.activation(
                            out=d[:], in_=p1[:],
                            func=mybir.ActivationFunctionType.Relu,
                            bias=bsteps[:, j, s:s + 1])
                        d1.append(d)
                    for j in range(KT):
                        p2 = ps.tile([P, CH], F32, space="PSUM", tag=f"mm2_{j}")
                        for k in range(KT):
                            nc.tensor.matmul(
                                p2[:], w2[:, k, j * P:(j + 1) * P], d1[k][:],
                                start=(k == 0), stop=(k == KT - 1))
                        nc.vector.tensor_add(
                            h[j][:, qs], h[j][:, qs].bitcast(F32), p2[:])
                        if not last:
                            nc.vector.tensor_reduce(
                                out=Snxt[j][:, gq],
                                in_=h[j][:, qs].bitcast(F32).rearrange(
                                    "p (g m) -> p g m", m=M),
                                axis=mybir.AxisListType.X,
                                op=mybir.AluOpType.add)
                    if last:
                        # logits for this chunk
                        pl = ps.tile([A, CH], F32, space="PSUM", tag="mm2_0",
                                     name="pl")
                        for k in range(KT):
                            nc.tensor.matmul(pl[:], wd[:, k, :], h[k][:, qs],
                                             start=(k == 0), stop=(k == KT - 1))
                        lg = lgt.tile([A, CH], F32, tag="lg")
                        nc.scalar.activation(
                            out=lg[:], in_=pl[:],
                            func=mybir.ActivationFunctionType.Identity,
                            bias=bd[:, 0:1])
                        nc.sync.dma_start(out=logT_d[:, qs], in_=lg[:])

    nc.compile()
    return nc


def _prep_inputs(agent_ids, emb, W1, b1, W2, b2, Wd, bd):
    agent_ids = np.asarray(agent_ids)
    emb = np.ascontiguousarray(np.asarray(emb, dtype=np.float32))
    W1 = np.asarray(W1, dtype=np.float32)
    b1 = np.asarray(b1, dtype=np.float32)
    W2 = np.asarray(W2, dtype=np.float32)
    b2 = np.asarray(b2, dtype=np.float32)
    Wd = np.asarray(Wd, dtype=np.float32)
    bd = np.asarray(bd, dtype=np.float32)

    W1h, W1c, W1h0 = W1[:H], W1[H:2 * H], W1[2 * H:]
    w1hp = W1h - INV * W1c
    w1ci = INV * W1c
    # b2 is never added on device: h' tracks h - s*b2.  Its effect on the
    # step-s pre-activation is s * b2 @ (W1h + W1c); on logits, 4 * b2 @ Wd.
    bb = b2 @ (W1h + W1c)
    bsteps = np.stack([s * bb for s in range(STEPS)], axis=1)  # [H, STEPS]
    bdp = bd + STEPS * (b2 @ Wd)

    def pack(w):  # [H, out] -> [P, KT, out]
        return np.ascontiguousarray(
            w.reshape(KT, P, w.shape[1]).transpose(1, 0, 2))

    def packb(b):  # [H] -> [P, KT]
        return np.ascontiguousarray(b.reshape(KT, P).T)

    shared = {
        "emb": emb,
        "w1hp": pack(w1hp),
        "w1h0": pack(W1h0),
        "w1ci": pack(w1ci),
        "w2": pack(W2),
        "wd": pack(Wd),
        "b1p": packb(b1),
        "bsteps": np.ascontiguousarray(
            bsteps.reshape(KT, P, STEPS).transpose(1, 0, 2)),
        "bdp": np.ascontiguousarray(bdp.reshape(A, 1)),
    }
    in_maps = []
    for c in range(NCORES):
        ids_local = np.asarray(
            agent_ids[c * G:(c + 1) * G], dtype=np.int32).reshape(T)
        ids_pt = np.ascontiguousarray(ids_local.reshape(T // P, P).T)
        in_maps.append({"ids_pt": ids_pt, **shared})
    return in_maps


def _run(in_maps, trace=False, tmpdir=None):
    from concourse.bass_utils import run_bass_kernel_spmd

    if "nc" not in _CACHE:
        _CACHE["nc"] = _build()
    nc = _CACHE["nc"]
    res = run_bass_kernel_spmd(
        nc, in_maps, core_ids=list(range(NCORES)), trace=trace, tmpdir=tmpdir)
    out = np.empty((B, M, A), dtype=np.float32)
    for c in range(NCORES):
        logT = res.results[c]["logT"]  # [A, T]
        out[c * G:(c + 1) * G] = logT.T.reshape(G, M, A)
    return out, res


def kernel(agent_ids, emb, W1, b1, W2, b2, Wd, bd):
    in_maps = _prep_inputs(agent_ids, emb, W1, b1, W2, b2, Wd, bd)
    out, _ = _run(in_maps, trace=False)
    return out



# revision 6
# speedup vs baseline: 1.0639x; 1.0639x over previous
"""CommNet forward on 8 TRN2 NeuronCores (Bass/Tile).

Model (per reference):
    h0 = emb[agent_ids]                      # (B, M, H)
    repeat 4x:
        c = (sum_m h - h) / (M-1)
        x = [h, c, h0]                       # (B, M, 3H)
        d = relu(x @ W1 + b1) @ W2 + b2
        h = h + d
    logits = h @ Wd + bd                     # (B, M, A)

Constants: B=1024, M=64, H=256, A=16, V=1000, 4 comm steps.

Sharding: data-parallel on B across 8 cores (128 groups / core); weights
replicated. Within a core every tensor is [hidden-on-partitions,
tokens-on-free] (tokens = group*64 + agent, T=8192 per core).

Pre-activation formulation (state = PRE in PSUM, never materialize h):
    P_0 = (W1h - inv*W1c + W1h0)^T h0 + (inv*W1c)^T segsum(h0)
    r_s = relu(P_s + b1 + s*bb)        bb = b2 @ (W1h + W1c)
    P_{s+1} = P_s + (W2 @ W1hp)^T r_s + bcast((W2 @ inv*W1c)^T segsum(r_s))
    logits = Wd^T h0 + (W2 @ Wd)^T (sum_s r_s) + (bd + 4 b2 @ Wd)
P accumulates IN PSUM across steps (matmul start=False accumulation);
groups of 64 tokens never cross a 512-token chunk, so the whole kernel is
a per-chunk pipeline (processed in pairs for cross-chunk engine overlap).
sum_s r_s accumulates on GPSIMD (idle otherwise).  Matmuls are float32r.
"""

import numpy as np

B, M, H, A, V = 1024, 64, 256, 16, 1000
STEPS = 4
NCORES = 8
G = B // NCORES          # groups per core = 128
T = G * M                # tokens per core = 8192
P = 128                  # partitions
KT = H // P              # K tiles per H = 2
CH = 512                 # tokens per chunk
NCH = T // CH            # chunks = 16
GPC = CH // M            # groups per chunk = 8
TPC = CH // P            # 128-token tiles per chunk = 4
INV = 1.0 / (M - 1)

GPSIMD_RSUM = True       # accumulate sum_s r_s on GPSIMD (else per-step PE)

_CACHE = {}


def _build():
    import concourse.bass as bass
    import concourse.tile as tile
    from concourse import bacc, mybir
    from concourse.masks import make_identity

    F32 = mybir.dt.float32
    F32R = mybir.dt.float32r
    I32 = mybir.dt.int32

    nc = bacc.Bacc("TRN2", target_bir_lowering=False, debug=False,
                   num_devices=NCORES)

    ids_d = nc.dram_tensor("ids_pt", [P, T // P], I32, kind="ExternalInput").ap()
    emb_d = nc.dram_tensor("emb", [V, H], F32, kind="ExternalInput").ap()
    wp0_d = nc.dram_tensor("wp0", [P, KT, H], F32, kind="ExternalInput").ap()
    m2_d = nc.dram_tensor("m2", [P, KT, H], F32, kind="ExternalInput").ap()
    mc_d = nc.dram_tensor("mc", [P, KT, H], F32, kind="ExternalInput").ap()
    w1ci_d = nc.dram_tensor("w1ci", [P, KT, H], F32, kind="ExternalInput").ap()
    wd_d = nc.dram_tensor("wd", [P, KT, A], F32, kind="ExternalInput").ap()
    md_d = nc.dram_tensor("md", [P, KT, A], F32, kind="ExternalInput").ap()
    bs_d = nc.dram_tensor("bsteps", [P, KT, STEPS], F32, kind="ExternalInput").ap()
    bd_d = nc.dram_tensor("bdp", [A, 1], F32, kind="ExternalInput").ap()
    logT_d = nc.dram_tensor("logT", [A, T], F32, kind="ExternalOutput").ap()

    with tile.TileContext(nc) as tc:
        with (
            tc.tile_pool(name="const", bufs=1) as const,
            tc.tile_pool(name="stage", bufs=2) as stage,
            tc.tile_pool(name="gat", bufs=8) as gat,
            tc.tile_pool(name="h0p", bufs=2) as h0p,
            tc.tile_pool(name="rp", bufs=2) as rp,
            tc.tile_pool(name="rsp", bufs=2) as rsp,
            tc.tile_pool(name="Rp", bufs=2) as Rpl,
            tc.tile_pool(name="swp", bufs=2) as swp,
            tc.tile_pool(name="lgt", bufs=2) as lgt,
            tc.tile_pool(name="prep", bufs=2, space="PSUM") as prep,
            tc.tile_pool(name="lgp", bufs=2, space="PSUM") as lgp,
            tc.tile_pool(name="scr", bufs=2, space="PSUM") as scr,
        ):
            # ---- constants / weights -------------------------------------
            ids = const.tile([P, T // P], I32)
            nc.sync.dma_start(out=ids[:], in_=ids_d[:])

            ident = const.tile([P, P], F32)
            make_identity(nc, ident[:])
            ident_r = const.tile([P, P], F32R)
            nc.vector.tensor_copy(ident_r[:], ident[:])

            mc = const.tile([P, KT, H], F32)
            nc.sync.dma_start(out=mc[:], in_=mc_d[:])
            w1ci = const.tile([P, KT, H], F32)
            nc.sync.dma_start(out=w1ci[:], in_=w1ci_d[:])
            bsteps = const.tile([P, KT, STEPS], F32)
            nc.sync.dma_start(out=bsteps[:], in_=bs_d[:])
            bd = const.tile([A, 1], F32)
            nc.sync.dma_start(out=bd[:], in_=bd_d[:])

            def load_r(dram, shape, name):
                st = stage.tile(shape, F32, tag="wstage", name=f"st_{name}")
                nc.sync.dma_start(out=st[:], in_=dram)
                t = const.tile(shape, F32R, tag=name, name=name)
                nc.vector.tensor_copy(t[:], st[:])
                return t

            wp0 = load_r(wp0_d[:], [P, KT, H], "wp0_r")
            m2 = load_r(m2_d[:], [P, KT, H], "m2_r")
            wd = load_r(wd_d[:], [P, KT, A], "wd_r")
            md = load_r(md_d[:], [P, KT, A], "md_r")

            NPAIR = NCH // 2

            def gather_pair(p):
                """Issue the 8 indirect gathers for pair p; return gst tiles."""
                gsts = []
                for ci in range(2):
                    q = 2 * p + ci
                    for tl in range(TPC):
                        t = q * TPC + tl
                        gst = gat.tile([P, H], F32, tag="gst",
                                       name=f"gst{q}_{tl}")
                        nc.gpsimd.indirect_dma_start(
                            out=gst[:],
                            out_offset=None,
                            in_=emb_d[:],
                            in_offset=bass.IndirectOffsetOnAxis(
                                ap=ids[:, t:t + 1], axis=0),
                        )
                        gsts.append(gst)
                return gsts

            def process_pair(p, gsts):
                qs_ = [2 * p, 2 * p + 1]
                # ---- per-chunk state tiles -------------------------------
                pre = {}    # (ci, j) -> psum tile [P, CH]
                h0c = {}    # (ci, k) -> sbuf tile [P, CH] f32r
                lg_ps = {}
                for ci in range(2):
                    for j in range(KT):
                        pre[ci, j] = prep.tile([P, CH], F32, space="PSUM",
                                               tag=f"pre{j}", name="pre")
                        h0c[ci, j] = h0p.tile([P, CH], F32R,
                                              tag=f"h0c{ci}{j}", name="h0c")

                # ---- transpose h0 into pre-psum staging, copy to SBUF ----
                for ci in range(2):
                    for tl in range(TPC):
                        gst = gsts[ci * TPC + tl]
                        for k in range(KT):
                            dst = pre[ci, k][:, tl * P:(tl + 1) * P]
                            nc.tensor.transpose(
                                out=dst, in_=gst[:, k * P:(k + 1) * P],
                                identity=ident[:])
                            hdst = h0c[ci, k][:, tl * P:(tl + 1) * P]
                            if (tl + k) % 2 == 0:
                                nc.vector.tensor_copy(hdst, dst)
                            else:
                                nc.scalar.activation(
                                    out=hdst, in_=dst,
                                    func=mybir.ActivationFunctionType.Identity)

                # ---- segsum(h0) for both chunks --------------------------
                R0 = Rpl.tile([P, KT, 2 * GPC], F32, tag="R", name="R0")
                for ci in range(2):
                    for k in range(KT):
                        nc.vector.tensor_reduce(
                            out=R0[:, k, ci * GPC:(ci + 1) * GPC],
                            in_=h0c[ci, k][:].bitcast(F32).rearrange(
                                "p (g m) -> p g m", m=M),
                            axis=mybir.AxisListType.X, op=mybir.AluOpType.add)

                def dsw_inject(Rt, wmat, start):
                    """psw = wmat^T @ Rt (both chunks), then broadcast-inject
                    into pre[ci,j]."""
                    psw = scr.tile([P, CH], F32, space="PSUM", tag="scr",
                                   name="psw")
                    for j in range(KT):
                        for k in range(KT):
                            nc.tensor.matmul(
                                psw[:, j * 2 * GPC:(j + 1) * 2 * GPC],
                                wmat[:, k, j * P:(j + 1) * P],
                                Rt[:, k, :],
                                start=(k == 0), stop=(k == KT - 1))
                    swd = swp.tile([P, KT, 2 * GPC], F32R, tag="swd",
                                   name="swd")
                    nc.vector.tensor_copy(
                        swd[:], psw[:, 0:KT * 2 * GPC].rearrange(
                            "p (j g) -> p j g", j=KT))
                    for ci in range(2):
                        for j in range(KT):
                            nc.tensor.matmul(
                                pre[ci, j][:].rearrange(
                                    "p (g m) -> p g m", g=GPC),
                                ident_r[:],
                                swd[:, j, ci * GPC:(ci + 1) * GPC]
                                .to_broadcast([P, GPC, M]),
                                start=False, stop=True,
                                skip_group_check=not start)

                # ---- P_0 = wp0^T h0 (+ SW_0 inject closes the group) -----
                for ci in range(2):
                    for j in range(KT):
                        for k in range(KT):
                            nc.tensor.matmul(
                                pre[ci, j][:], wp0[:, k, j * P:(j + 1) * P],
                                h0c[ci, k][:],
                                start=(k == 0), stop=False)
                dsw_inject(R0, w1ci, start=True)

                # ---- comm steps ------------------------------------------
                rsum = {}
                for s in range(STEPS):
                    last = s == STEPS - 1
                    r = {}
                    for ci in range(2):
                        for j in range(KT):
                            rt = rp.tile([P, CH], F32R, tag=f"r{ci}{j}",
                                         name="r")
                            nc.scalar.activation(
                                out=rt[:], in_=pre[ci, j][:],
                                func=mybir.ActivationFunctionType.Relu,
                                bias=bsteps[:, j, s:s + 1])
                            r[ci, j] = rt
                    # rsum accumulation (gpsimd; SBUF only)
                    for ci in range(2):
                        for j in range(KT):
                            if s == 0:
                                rs = rsp.tile([P, CH], F32R, tag=f"rs{ci}{j}",
                                              name="rsum")
                                rsum[ci, j] = rs
                                nc.gpsimd.tensor_copy(rs[:], r[ci, j][:])
                            else:
                                nc.gpsimd.tensor_add(
                                    rsum[ci, j][:], rsum[ci, j][:],
                                    r[ci, j][:])
                    if last:
                        break
                    # segsum(r) for both chunks
                    Rt = Rpl.tile([P, KT, 2 * GPC], F32, tag="R", name="Rt")
                    for ci in range(2):
                        for k in range(KT):
                            nc.vector.tensor_reduce(
                                out=Rt[:, k, ci * GPC:(ci + 1) * GPC],
                                in_=r[ci, k][:].bitcast(F32).rearrange(
                                    "p (g m) -> p g m", m=M),
                                axis=mybir.AxisListType.X,
                                op=mybir.AluOpType.add)
                    # pre += m2^T r   (cross-step accumulate, group ended)
                    for ci in range(2):
                        for j in range(KT):
                            for k in range(KT):
                                nc.tensor.matmul(
                                    pre[ci, j][:], m2[:, k, j * P:(j + 1) * P],
                                    r[ci, k][:],
                                    start=False, stop=False,
                                    skip_group_check=True)
                    # pre += bcast(mc^T segsum(r))
                    dsw_inject(Rt, mc, start=False)

                # ---- logits: Wd^T h0 + md^T rsum + bdp -------------------
                for ci in range(2):
                    pl = lgp.tile([A, CH], F32, space="PSUM", tag="lg",
                                  name="pl")
                    for k in range(KT):
                        nc.tensor.matmul(pl[:], wd[:, k, :], h0c[ci, k][:],
                                         start=(k == 0), stop=False)
                    for k in range(KT):
                        nc.tensor.matmul(pl[:], md[:, k, :], rsum[ci, k][:],
                                         start=False, stop=(k == KT - 1))
                    lg = lgt.tile([A, CH], F32, tag="lg")
                    nc.scalar.activation(
                        out=lg[:], in_=pl[:],
                        func=mybir.ActivationFunctionType.Identity,
                        bias=bd[:, 0:1])
                    q = qs_[ci]
                    nc.sync.dma_start(
                        out=logT_d[:, q * CH:(q + 1) * CH], in_=lg[:])

            gsts = gather_pair(0)
            for p in range(NPAIR):
                nxt = gather_pair(p + 1) if p + 1 < NPAIR else None
                process_pair(p, gsts)
                gsts = nxt

    nc.compile()
    return nc


def _prep_inputs(agent_ids, emb, W1, b1, W2, b2, Wd, bd):
    agent_ids = np.asarray(agent_ids)
    emb = np.ascontiguousarray(np.asarray(emb, dtype=np.float32))
    W1 = np.asarray(W1, dtype=np.float32)
    b1 = np.asarray(b1, dtype=np.float32)
    W2 = np.asarray(W2, dtype=np.float32)
    b2 = np.asarray(b2, dtype=np.float32)
    Wd = np.asarray(Wd, dtype=np.float32)
    bd = np.asarray(bd, dtype=np.float32)

    W1h, W1c, W1h0 = W1[:H], W1[H:2 * H], W1[2 * H:]
    w1hp = W1h - INV * W1c
    w1ci = INV * W1c
    wp0 = w1hp + W1h0
    m2 = W2 @ w1hp
    mc = W2 @ w1ci
    md = W2 @ Wd
    bb = b2 @ (W1h + W1c)
    # r_s = relu(P_s + b1 + s*bb)
    bsteps = np.stack([b1 + s * bb for s in range(STEPS)], axis=1)  # [H, S]
    bdp = bd + STEPS * (b2 @ Wd)

    def pack(w):  # [H, out] -> [P, KT, out]
        return np.ascontiguousarray(
            w.reshape(KT, P, w.shape[1]).transpose(1, 0, 2))

    shared = {
        "emb": emb,
        "wp0": pack(wp0),
        "m2": pack(m2),
        "mc": pack(mc),
        "w1ci": pack(w1ci),
        "wd": pack(Wd),
        "md": pack(md),
        "bsteps": np.ascontiguousarray(
            bsteps.reshape(KT, P, STEPS).transpose(1, 0, 2)),
        "bdp": np.ascontiguousarray(bdp.reshape(A, 1)),
    }
    in_maps = []
    for c in range(NCORES):
        ids_local = np.asarray(
            agent_ids[c * G:(c + 1) * G], dtype=np.int32).reshape(T)
        ids_pt = np.ascontiguousarray(ids_local.reshape(T // P, P).T)
        in_maps.append({"ids_pt": ids_pt, **shared})
    return in_maps


def _run(in_maps, trace=False, tmpdir=None):
    from concourse.bass_utils import run_bass_kernel_spmd

    if "nc" not in _CACHE:
        _CACHE["nc"] = _build()
    nc = _CACHE["nc"]
    res = run_bass_kernel_spmd(
        nc, in_maps, core_ids=list(range(NCORES)), trace=trace, tmpdir=tmpdir)
    out = np.empty((B, M, A), dtype=np.float32)
    for c in range(NCORES):
        logT = res.results[c]["logT"]  # [A, T]
        out[c * G:(c + 1) * G] = logT.T.reshape(G, M, A)
    return out, res


def kernel(agent_ids, emb, W1, b1, W2, b2, Wd, bd):
    in_maps = _prep_inputs(agent_ids, emb, W1, b1, W2, b2, Wd, bd)
    out, _ = _run(in_maps, trace=False)
    return out


# revision 9
# speedup vs baseline: 1.3201x; 1.2409x over previous
"""CommNet forward on 8 TRN2 NeuronCores (Bass/Tile).

Model (per reference):
    h0 = emb[agent_ids]                      # (B, M, H)
    repeat 4x:
        c = (sum_m h - h) / (M-1)
        x = [h, c, h0]                       # (B, M, 3H)
        d = relu(x @ W1 + b1) @ W2 + b2
        h = h + d
    logits = h @ Wd + bd                     # (B, M, A)

Constants: B=1024, M=64, H=256, A=16, V=1000, 4 comm steps.

Sharding: data-parallel on B across 8 cores (128 groups / core); weights
replicated. Within a core every tensor is [hidden-on-partitions,
tokens-on-free] (tokens = group*64 + agent, T=8192 per core).

Pre-activation formulation (state = PRE in PSUM, never materialize h):
    P_0 = (W1h - inv*W1c + W1h0)^T h0 + (inv*W1c)^T segsum(h0)
    r_s = relu(P_s + b1 + s*bb)        bb = b2 @ (W1h + W1c)
    P_{s+1} = P_s + (W2 @ W1hp)^T r_s + bcast((W2 @ inv*W1c)^T segsum(r_s))
    logits = Wd^T h0 + (W2 @ Wd)^T (sum_s r_s) + (bd + 4 b2 @ Wd)
P accumulates IN PSUM across steps (matmul start=False accumulation);
groups of 64 tokens never cross a 512-token chunk, so the whole kernel is
a per-chunk pipeline (processed in pairs for cross-chunk engine overlap).
sum_s r_s accumulates into the logits PSUM bank per step.  Matmuls are float32r.
"""

import numpy as np

B, M, H, A, V = 1024, 64, 256, 16, 1000
STEPS = 4
NCORES = 8
G = B // NCORES          # groups per core = 128
T = G * M                # tokens per core = 8192
P = 128                  # partitions
KT = H // P              # K tiles per H = 2
CH = 512                 # tokens per chunk
NCH = T // CH            # chunks = 16
GPC = CH // M            # groups per chunk = 8
TPC = CH // P            # 128-token tiles per chunk = 4
INV = 1.0 / (M - 1)

_CACHE = {}


def _build():
    import concourse.bass as bass
    import concourse.tile as tile
    from concourse import bacc, mybir
    from concourse.masks import make_identity

    F32 = mybir.dt.float32
    F32R = mybir.dt.float32r
    I32 = mybir.dt.int32

    nc = bacc.Bacc("TRN2", target_bir_lowering=False, debug=False,
                   num_devices=NCORES)

    ids_d = nc.dram_tensor("ids_pt", [P, T // P], I32, kind="ExternalInput").ap()
    emb_d = nc.dram_tensor("emb", [V, H], F32, kind="ExternalInput").ap()
    wp0_d = nc.dram_tensor("wp0", [P, KT, H], F32, kind="ExternalInput").ap()
    m2_d = nc.dram_tensor("m2", [P, KT, H], F32, kind="ExternalInput").ap()
    mc_d = nc.dram_tensor("mc", [P, KT, H], F32, kind="ExternalInput").ap()
    w1ci_d = nc.dram_tensor("w1ci", [P, KT, H], F32, kind="ExternalInput").ap()
    wd_d = nc.dram_tensor("wd", [P, KT, A], F32, kind="ExternalInput").ap()
    md_d = nc.dram_tensor("md", [P, KT, A], F32, kind="ExternalInput").ap()
    bs_d = nc.dram_tensor("bsteps", [P, KT, STEPS], F32, kind="ExternalInput").ap()
    bd_d = nc.dram_tensor("bdp", [A, 1], F32, kind="ExternalInput").ap()
    logT_d = nc.dram_tensor("logT", [A, T], F32, kind="ExternalOutput").ap()

    with tile.TileContext(nc) as tc:
        with (
            tc.tile_pool(name="const", bufs=1) as const,
            tc.tile_pool(name="stage", bufs=2) as stage,
            tc.tile_pool(name="gat", bufs=8) as gat,
            tc.tile_pool(name="h0p", bufs=2) as h0p,
            tc.tile_pool(name="rp", bufs=2) as rp,
            tc.tile_pool(name="Rp", bufs=2) as Rpl,
            tc.tile_pool(name="swp", bufs=2) as swp,
            tc.tile_pool(name="lgt", bufs=2) as lgt,
            tc.tile_pool(name="prep", bufs=2, space="PSUM") as prep,
            tc.tile_pool(name="lgp", bufs=2, space="PSUM") as lgp,
            tc.tile_pool(name="scr", bufs=2, space="PSUM") as scr,
        ):
            # ---- constants / weights -------------------------------------
            ids = const.tile([P, T // P], I32)
            nc.sync.dma_start(out=ids[:], in_=ids_d[:])

            ident = const.tile([P, P], F32)
            make_identity(nc, ident[:])
            ident_r = const.tile([P, P], F32R)
            nc.vector.tensor_copy(ident_r[:], ident[:])

            mc = const.tile([P, KT, H], F32)
            nc.sync.dma_start(out=mc[:], in_=mc_d[:])
            w1ci = const.tile([P, KT, H], F32)
            nc.sync.dma_start(out=w1ci[:], in_=w1ci_d[:])
            bsteps = const.tile([P, KT, STEPS], F32)
            nc.sync.dma_start(out=bsteps[:], in_=bs_d[:])
            bd = const.tile([A, 1], F32)
            nc.sync.dma_start(out=bd[:], in_=bd_d[:])

            def load_r(dram, shape, name):
                st = stage.tile(shape, F32, tag="wstage", name=f"st_{name}")
                nc.sync.dma_start(out=st[:], in_=dram)
                t = const.tile(shape, F32R, tag=name, name=name)
                nc.vector.tensor_copy(t[:], st[:])
                return t

            wp0 = load_r(wp0_d[:], [P, KT, H], "wp0_r")
            m2 = load_r(m2_d[:], [P, KT, H], "m2_r")
            wd = load_r(wd_d[:], [P, KT, A], "wd_r")
            md = load_r(md_d[:], [P, KT, A], "md_r")

            NPAIR = NCH // 2

            def gather_pair(p):
                """Issue the 8 indirect gathers for pair p; return gst tiles."""
                gsts = []
                for ci in range(2):
                    q = 2 * p + ci
                    for tl in range(TPC):
                        t = q * TPC + tl
                        gst = gat.tile([P, H], F32, tag="gst",
                                       name=f"gst{q}_{tl}")
                        nc.gpsimd.indirect_dma_start(
                            out=gst[:],
                            out_offset=None,
                            in_=emb_d[:],
                            in_offset=bass.IndirectOffsetOnAxis(
                                ap=ids[:, t:t + 1], axis=0),
                        )
                        gsts.append(gst)
                return gsts

            def process_pair(p, gsts):
                qs_ = [2 * p, 2 * p + 1]
                # ---- per-chunk state tiles -------------------------------
                pre = {}    # (ci, j) -> psum tile [P, CH]
                h0c = {}    # (ci, k) -> sbuf tile [P, CH] f32r
                lg_ps = {}
                for ci in range(2):
                    for j in range(KT):
                        pre[ci, j] = prep.tile([P, CH], F32, space="PSUM",
                                               tag=f"pre{j}", name="pre")
                        h0c[ci, j] = h0p.tile([P, CH], F32R,
                                              tag=f"h0c{ci}{j}", name="h0c")

                # ---- transpose h0 into pre-psum staging, copy to SBUF ----
                for ci in range(2):
                    for tl in range(TPC):
                        gst = gsts[ci * TPC + tl]
                        for k in range(KT):
                            dst = pre[ci, k][:, tl * P:(tl + 1) * P]
                            nc.tensor.transpose(
                                out=dst, in_=gst[:, k * P:(k + 1) * P],
                                identity=ident[:])
                            hdst = h0c[ci, k][:, tl * P:(tl + 1) * P]
                            if (tl + k) % 2 == 0:
                                nc.vector.tensor_copy(hdst, dst)
                            else:
                                nc.scalar.activation(
                                    out=hdst, in_=dst,
                                    func=mybir.ActivationFunctionType.Identity)

                # ---- segsum(h0) for both chunks --------------------------
                R0 = Rpl.tile([P, KT, 2 * GPC], F32, tag="R", name="R0")
                for ci in range(2):
                    for k in range(KT):
                        nc.vector.tensor_reduce(
                            out=R0[:, k, ci * GPC:(ci + 1) * GPC],
                            in_=h0c[ci, k][:].bitcast(F32).rearrange(
                                "p (g m) -> p g m", m=M),
                            axis=mybir.AxisListType.X, op=mybir.AluOpType.add)

                def dsw_inject(Rt, wmat, start):
                    """psw = wmat^T @ Rt (both chunks), then broadcast-inject
                    into pre[ci,j]."""
                    psw = scr.tile([P, CH], F32, space="PSUM", tag="scr",
                                   name="psw")
                    for j in range(KT):
                        for k in range(KT):
                            nc.tensor.matmul(
                                psw[:, j * 2 * GPC:(j + 1) * 2 * GPC],
                                wmat[:, k, j * P:(j + 1) * P],
                                Rt[:, k, :],
                                start=(k == 0), stop=(k == KT - 1))
                    swd = swp.tile([P, KT, 2 * GPC], F32R, tag="swd",
                                   name="swd")
                    nc.vector.tensor_copy(
                        swd[:], psw[:, 0:KT * 2 * GPC].rearrange(
                            "p (j g) -> p j g", j=KT))
                    for ci in range(2):
                        for j in range(KT):
                            nc.tensor.matmul(
                                pre[ci, j][:].rearrange(
                                    "p (g m) -> p g m", g=GPC),
                                ident_r[:],
                                swd[:, j, ci * GPC:(ci + 1) * GPC]
                                .to_broadcast([P, GPC, M]),
                                start=False, stop=True,
                                skip_group_check=not start)

                # ---- P_0 = wp0^T h0 (+ SW_0 inject closes the group) -----
                for ci in range(2):
                    for j in range(KT):
                        for k in range(KT):
                            nc.tensor.matmul(
                                pre[ci, j][:], wp0[:, k, j * P:(j + 1) * P],
                                h0c[ci, k][:],
                                start=(k == 0), stop=False)
                dsw_inject(R0, w1ci, start=True)

                # ---- logits psum: init with Wd^T h0 ----------------------
                lgps = {}
                for ci in range(2):
                    pl = lgp.tile([A, CH], F32, space="PSUM", tag="lg",
                                  name="pl")
                    lgps[ci] = pl
                    for k in range(KT):
                        nc.tensor.matmul(pl[:], wd[:, k, :], h0c[ci, k][:],
                                         start=(k == 0), stop=(k == KT - 1))

                # ---- comm steps ------------------------------------------
                for s in range(STEPS):
                    last = s == STEPS - 1
                    r = {}
                    for ci in range(2):
                        for j in range(KT):
                            rt = rp.tile([P, CH], F32R, tag=f"r{ci}{j}",
                                         name="r")
                            nc.scalar.activation(
                                out=rt[:], in_=pre[ci, j][:],
                                func=mybir.ActivationFunctionType.Relu,
                                bias=bsteps[:, j, s:s + 1])
                            r[ci, j] = rt
                    # logits += md^T r_s   (cross-step psum accumulate)
                    for ci in range(2):
                        for k in range(KT):
                            nc.tensor.matmul(
                                lgps[ci][:], md[:, k, :], r[ci, k][:],
                                start=False, stop=False,
                                skip_group_check=True)
                    if last:
                        break
                    # segsum(r) for both chunks
                    Rt = Rpl.tile([P, KT, 2 * GPC], F32, tag="R", name="Rt")
                    for ci in range(2):
                        for k in range(KT):
                            nc.vector.tensor_reduce(
                                out=Rt[:, k, ci * GPC:(ci + 1) * GPC],
                                in_=r[ci, k][:].bitcast(F32).rearrange(
                                    "p (g m) -> p g m", m=M),
                                axis=mybir.AxisListType.X,
                                op=mybir.AluOpType.add)
                    # pre += m2^T r   (cross-step accumulate, group ended)
                    for ci in range(2):
                        for j in range(KT):
                            for k in range(KT):
                                nc.tensor.matmul(
                                    pre[ci, j][:], m2[:, k, j * P:(j + 1) * P],
                                    r[ci, k][:],
                                    start=False, stop=False,
                                    skip_group_check=True)
                    # pre += bcast(mc^T segsum(r))
                    dsw_inject(Rt, mc, start=False)

                # ---- logits out ------------------------------------------
                for ci in range(2):
                    lg = lgt.tile([A, CH], F32, tag="lg")
                    nc.scalar.activation(
                        out=lg[:], in_=lgps[ci][:],
                        func=mybir.ActivationFunctionType.Identity,
                        bias=bd[:, 0:1])
                    q = qs_[ci]
                    nc.sync.dma_start(
                        out=logT_d[:, q * CH:(q + 1) * CH], in_=lg[:])

            gsts = gather_pair(0)
            for p in range(NPAIR):
                nxt = gather_pair(p + 1) if p + 1 < NPAIR else None
                process_pair(p, gsts)
                gsts = nxt

    nc.compile()
    return nc


def _prep_inputs(agent_ids, emb, W1, b1, W2, b2, Wd, bd):
    agent_ids = np.asarray(agent_ids)
    emb = np.ascontiguousarray(np.asarray(emb, dtype=np.float32))
    W1 = np.asarray(W1, dtype=np.float32)
    b1 = np.asarray(b1, dtype=np.float32)
    W2 = np.asarray(W2, dtype=np.float32)
    b2 = np.asarray(b2, dtype=np.float32)
    Wd = np.asarray(Wd, dtype=np.float32)
    bd = np.asarray(bd, dtype=np.float32)

    W1h, W1c, W1h0 = W1[:H], W1[H:2 * H], W1[2 * H:]
    w1hp = W1h - INV * W1c
    w1ci = INV * W1c
    wp0 = w1hp + W1h0
    m2 = W2 @ w1hp
    mc = W2 @ w1ci
    md = W2 @ Wd
    bb = b2 @ (W1h + W1c)
    # r_s = relu(P_s + b1 + s*bb)
    bsteps = np.stack([b1 + s * bb for s in range(STEPS)], axis=1)  # [H, S]
    bdp = bd + STEPS * (b2 @ Wd)

    def pack(w):  # [H, out] -> [P, KT, out]
        return np.ascontiguousarray(
            w.reshape(KT, P, w.shape[1]).transpose(1, 0, 2))

    shared = {
        "emb": emb,
        "wp0": pack(wp0),
        "m2": pack(m2),
        "mc": pack(mc),
        "w1ci": pack(w1ci),
        "wd": pack(Wd),
        "md": pack(md),
        "bsteps": np.ascontiguousarray(
            bsteps.reshape(KT, P, STEPS).transpose(1, 0, 2)),
        "bdp": np.ascontiguousarray(bdp.reshape(A, 1)),
    }
    in_maps = []
    for c in range(NCORES):
        ids_local = np.asarray(
            agent_ids[c * G:(c + 1) * G], dtype=np.int32).reshape(T)
        ids_pt = np.ascontiguousarray(ids_local.reshape(T // P, P).T)
        in_maps.append({"ids_pt": ids_pt, **shared})
    return in_maps


def _run(in_maps, trace=False, tmpdir=None):
    from concourse.bass_utils import run_bass_kernel_spmd

    if "nc" not in _CACHE:
        _CACHE["nc"] = _build()
    nc = _CACHE["nc"]
    res = run_bass_kernel_spmd(
        nc, in_maps, core_ids=list(range(NCORES)), trace=trace, tmpdir=tmpdir)
    out = np.empty((B, M, A), dtype=np.float32)
    for c in range(NCORES):
        logT = res.results[c]["logT"]  # [A, T]
        out[c * G:(c + 1) * G] = logT.T.reshape(G, M, A)
    return out, res


def kernel(agent_ids, emb, W1, b1, W2, b2, Wd, bd):
    in_maps = _prep_inputs(agent_ids, emb, W1, b1, W2, b2, Wd, bd)
    out, _ = _run(in_maps, trace=False)
    return out


# revision 11
# speedup vs baseline: 1.3278x; 1.0058x over previous
"""CommNet forward on 8 TRN2 NeuronCores (Bass/Tile).

Model (per reference):
    h0 = emb[agent_ids]                      # (B, M, H)
    repeat 4x:
        c = (sum_m h - h) / (M-1)
        x = [h, c, h0]                       # (B, M, 3H)
        d = relu(x @ W1 + b1) @ W2 + b2
        h = h + d
    logits = h @ Wd + bd                     # (B, M, A)

Constants: B=1024, M=64, H=256, A=16, V=1000, 4 comm steps.

Sharding: data-parallel on B across 8 cores (128 groups / core); weights
replicated. Within a core every tensor is [hidden-on-partitions,
tokens-on-free] (tokens = group*64 + agent, T=8192 per core).

Pre-activation formulation (state = PRE in PSUM, never materialize h):
    P_0 = (W1h - inv*W1c + W1h0)^T h0 + (inv*W1c)^T segsum(h0)
    r_s = relu(P_s + b1 + s*bb)        bb = b2 @ (W1h + W1c)
    P_{s+1} = P_s + (W2 @ W1hp)^T r_s + bcast((W2 @ inv*W1c)^T segsum(r_s))
    logits = Wd^T h0 + (W2 @ Wd)^T (sum_s r_s) + (bd + 4 b2 @ Wd)
P accumulates IN PSUM (fp32) across steps via matmul start=False; groups of
64 tokens never cross a 512-token chunk, so the kernel is a per-chunk
pipeline (chunk pairs interleaved for engine overlap).  sum_s r_s
accumulates into the logits PSUM bank per step.  All matmul operands are
BF16 (separate fast-loadable weights; fp32/f32r pay a serial inline weight
load per matmul); PSUM accumulation stays fp32.
"""

import numpy as np
import ml_dtypes

BF = ml_dtypes.bfloat16

B, M, H, A, V = 1024, 64, 256, 16, 1000
STEPS = 4
NCORES = 8
G = B // NCORES          # groups per core = 128
T = G * M                # tokens per core = 8192
P = 128                  # partitions
KT = H // P              # K tiles per H = 2
CH = 512                 # tokens per chunk
NCH = T // CH            # chunks = 16
GPC = CH // M            # groups per chunk = 8
TPC = CH // P            # 128-token tiles per chunk = 4
INV = 1.0 / (M - 1)

_CACHE = {}


def _build():
    import concourse.bass as bass
    import concourse.tile as tile
    from concourse import bacc, mybir
    from concourse.masks import make_identity

    F32 = mybir.dt.float32
    BF16 = mybir.dt.bfloat16
    I32 = mybir.dt.int32

    nc = bacc.Bacc("TRN2", target_bir_lowering=False, debug=False,
                   num_devices=NCORES)

    ids_d = nc.dram_tensor("ids_pt", [P, T // P], I32, kind="ExternalInput").ap()
    emb_d = nc.dram_tensor("emb", [V, H], BF16, kind="ExternalInput").ap()
    wp0_d = nc.dram_tensor("wp0", [P, KT, H], BF16, kind="ExternalInput").ap()
    m2_d = nc.dram_tensor("m2", [P, KT, H], BF16, kind="ExternalInput").ap()
    mc_d = nc.dram_tensor("mc", [P, KT, H], BF16, kind="ExternalInput").ap()
    w1ci_d = nc.dram_tensor("w1ci", [P, KT, H], BF16, kind="ExternalInput").ap()
    wd_d = nc.dram_tensor("wd", [P, KT, A], BF16, kind="ExternalInput").ap()
    md_d = nc.dram_tensor("md", [P, KT, A], BF16, kind="ExternalInput").ap()
    bs_d = nc.dram_tensor("bsteps", [P, KT, STEPS], F32, kind="ExternalInput").ap()
    bd_d = nc.dram_tensor("bdp", [A, 1], F32, kind="ExternalInput").ap()
    logT_d = nc.dram_tensor("logT", [A, T], F32, kind="ExternalOutput").ap()

    with tile.TileContext(nc) as tc:
        with (
            tc.tile_pool(name="const", bufs=1) as const,
            tc.tile_pool(name="gat", bufs=8) as gat,
            tc.tile_pool(name="h0p", bufs=2) as h0p,
            tc.tile_pool(name="rp", bufs=2) as rp,
            tc.tile_pool(name="Rp", bufs=2) as Rpl,
            tc.tile_pool(name="swp", bufs=2) as swp,
            tc.tile_pool(name="lgt", bufs=2) as lgt,
            tc.tile_pool(name="prep", bufs=2, space="PSUM") as prep,
            tc.tile_pool(name="lgp", bufs=2, space="PSUM") as lgp,
            tc.tile_pool(name="scr", bufs=2, space="PSUM") as scr,
        ):
            # ---- constants / weights -------------------------------------
            ids = const.tile([P, T // P], I32)
            nc.sync.dma_start(out=ids[:], in_=ids_d[:])

            identf = const.tile([P, P], F32)
            make_identity(nc, identf[:])
            ident = const.tile([P, P], BF16)
            nc.vector.tensor_copy(ident[:], identf[:])

            wp0 = const.tile([P, KT, H], BF16, name="wp0")
            nc.sync.dma_start(out=wp0[:], in_=wp0_d[:])
            m2 = const.tile([P, KT, H], BF16, name="m2")
            nc.sync.dma_start(out=m2[:], in_=m2_d[:])
            mc = const.tile([P, KT, H], BF16, name="mc")
            nc.sync.dma_start(out=mc[:], in_=mc_d[:])
            w1ci = const.tile([P, KT, H], BF16, name="w1ci")
            nc.sync.dma_start(out=w1ci[:], in_=w1ci_d[:])
            wd = const.tile([P, KT, A], BF16, name="wd")
            nc.sync.dma_start(out=wd[:], in_=wd_d[:])
            md = const.tile([P, KT, A], BF16, name="md")
            nc.sync.dma_start(out=md[:], in_=md_d[:])
            bsteps = const.tile([P, KT, STEPS], F32)
            nc.sync.dma_start(out=bsteps[:], in_=bs_d[:])
            bd = const.tile([A, 1], F32)
            nc.sync.dma_start(out=bd[:], in_=bd_d[:])

            NPAIR = NCH // 2

            def gather_pair(p):
                """Issue the 8 indirect gathers for pair p; return gst tiles."""
                gsts = []
                for ci in range(2):
                    q = 2 * p + ci
                    for tl in range(TPC):
                        t = q * TPC + tl
                        gst = gat.tile([P, H], BF16, tag="gst",
                                       name=f"gst{q}_{tl}")
                        nc.gpsimd.indirect_dma_start(
                            out=gst[:],
                            out_offset=None,
                            in_=emb_d[:],
                            in_offset=bass.IndirectOffsetOnAxis(
                                ap=ids[:, t:t + 1], axis=0),
                        )
                        gsts.append(gst)
                return gsts

            def process_pair(p, gsts):
                # ---- per-chunk state tiles -------------------------------
                pre = {}    # (ci, j) -> psum tile [P, CH] fp32
                h0c = {}    # (ci, k) -> sbuf tile [P, CH] bf16
                for ci in range(2):
                    for j in range(KT):
                        pre[ci, j] = prep.tile([P, CH], F32, space="PSUM",
                                               tag=f"pre{j}", name="pre")
                        h0c[ci, j] = h0p.tile([P, CH], BF16,
                                              tag=f"h0c{ci}{j}", name="h0c")

                # ---- transpose h0 (PE) via scr psum, copy to SBUF --------
                for ci in range(2):
                    for tl in range(TPC):
                        gst = gsts[ci * TPC + tl]
                        for k in range(KT):
                            pt = scr.tile([P, P], BF16, space="PSUM",
                                          tag="scr", bufs=2, name="pt")
                            nc.tensor.transpose(
                                out=pt[:], in_=gst[:, k * P:(k + 1) * P],
                                identity=ident[:])
                            hdst = h0c[ci, k][:, tl * P:(tl + 1) * P]
                            if (tl + k) % 2 == 0:
                                nc.vector.tensor_copy(hdst, pt[:])
                            else:
                                nc.scalar.activation(
                                    out=hdst, in_=pt[:],
                                    func=mybir.ActivationFunctionType.Identity)

                # ---- segsum(h0) for both chunks --------------------------
                R0 = Rpl.tile([P, KT, 2 * GPC], BF16, tag="R", name="R0")
                with nc.allow_low_precision(reason="segsum of 64 bf16 vals"):
                    for ci in range(2):
                        for k in range(KT):
                            nc.vector.tensor_reduce(
                                out=R0[:, k, ci * GPC:(ci + 1) * GPC],
                                in_=h0c[ci, k][:].rearrange(
                                    "p (g m) -> p g m", m=M),
                                axis=mybir.AxisListType.X,
                                op=mybir.AluOpType.add)

                def dsw_inject(Rt, wmat, start):
                    """psw = wmat^T @ Rt (both chunks), broadcast-inject into
                    pre[ci,j]."""
                    psw = scr.tile([P, KT * 2 * GPC], F32, space="PSUM",
                                   tag="scr", bufs=2, name="psw")
                    for j in range(KT):
                        for k in range(KT):
                            nc.tensor.matmul(
                                psw[:, j * 2 * GPC:(j + 1) * 2 * GPC],
                                wmat[:, k, j * P:(j + 1) * P],
                                Rt[:, k, :],
                                start=(k == 0), stop=(k == KT - 1))
                    swd = swp.tile([P, KT, 2 * GPC], BF16, tag="swd",
                                   name="swd")
                    nc.vector.tensor_copy(
                        swd[:], psw[:].rearrange("p (j g) -> p j g", j=KT))
                    for ci in range(2):
                        for j in range(KT):
                            nc.tensor.matmul(
                                pre[ci, j][:].rearrange(
                                    "p (g m) -> p g m", g=GPC),
                                ident[:],
                                swd[:, j, ci * GPC:(ci + 1) * GPC]
                                .to_broadcast([P, GPC, M]),
                                start=False, stop=True,
                                skip_group_check=not start)

                # ---- P_0 = wp0^T h0 (+ SW_0 inject closes the group) -----
                for ci in range(2):
                    for j in range(KT):
                        for k in range(KT):
                            nc.tensor.matmul(
                                pre[ci, j][:], wp0[:, k, j * P:(j + 1) * P],
                                h0c[ci, k][:],
                                start=(k == 0), stop=False)
                dsw_inject(R0, w1ci, start=True)

                # ---- logits psum: init with Wd^T h0 ----------------------
                lgps = {}
                for ci in range(2):
                    pl = lgp.tile([A, CH], F32, space="PSUM", tag="lg",
                                  name="pl")
                    lgps[ci] = pl
                    for k in range(KT):
                        nc.tensor.matmul(pl[:], wd[:, k, :], h0c[ci, k][:],
                                         start=(k == 0), stop=(k == KT - 1))

                # ---- comm steps ------------------------------------------
                for s in range(STEPS):
                    last = s == STEPS - 1
                    r = {}
                    for ci in range(2):
                        for j in range(KT):
                            rt = rp.tile([P, CH], BF16, tag=f"r{ci}{j}",
                                         name="r")
                            nc.scalar.activation(
                                out=rt[:], in_=pre[ci, j][:],
                                func=mybir.ActivationFunctionType.Relu,
                                bias=bsteps[:, j, s:s + 1])
                            r[ci, j] = rt
                    # logits += md^T r_s   (cross-step psum accumulate)
                    for ci in range(2):
                        for k in range(KT):
                            nc.tensor.matmul(
                                lgps[ci][:], md[:, k, :], r[ci, k][:],
                                start=False, stop=False,
                                skip_group_check=True)
                    if last:
                        break
                    # segsum(r) for both chunks
                    Rt = Rpl.tile([P, KT, 2 * GPC], BF16, tag="R", name="Rt")
                    with nc.allow_low_precision(
                            reason="segsum of 64 bf16 vals"):
                        for ci in range(2):
                            for k in range(KT):
                                nc.vector.tensor_reduce(
                                    out=Rt[:, k, ci * GPC:(ci + 1) * GPC],
                                    in_=r[ci, k][:].rearrange(
                                        "p (g m) -> p g m", m=M),
                                    axis=mybir.AxisListType.X,
                                    op=mybir.AluOpType.add)
                    # pre += m2^T r   (cross-step accumulate, group ended)
                    for ci in range(2):
                        for j in range(KT):
                            for k in range(KT):
                                nc.tensor.matmul(
                                    pre[ci, j][:], m2[:, k, j * P:(j + 1) * P],
                                    r[ci, k][:],
                                    start=False, stop=False,
                                    skip_group_check=True)
                    # pre += bcast(mc^T segsum(r))
                    dsw_inject(Rt, mc, start=False)

                # ---- logits out ------------------------------------------
                for ci in range(2):
                    lg = lgt.tile([A, CH], F32, tag="lg")
                    nc.scalar.activation(
                        out=lg[:], in_=lgps[ci][:],
                        func=mybir.ActivationFunctionType.Identity,
                        bias=bd[:, 0:1])
                    q = 2 * p + ci
                    nc.sync.dma_start(
                        out=logT_d[:, q * CH:(q + 1) * CH], in_=lg[:])

            gsts = gather_pair(0)
            for p in range(NPAIR):
                nxt = gather_pair(p + 1) if p + 1 < NPAIR else None
                process_pair(p, gsts)
                gsts = nxt

    nc.compile()
    return nc


def _prep_inputs(agent_ids, emb, W1, b1, W2, b2, Wd, bd):
    agent_ids = np.asarray(agent_ids)
    emb = np.asarray(emb, dtype=np.float32)
    W1 = np.asarray(W1, dtype=np.float32)
    b1 = np.asarray(b1, dtype=np.float32)
    W2 = np.asarray(W2, dtype=np.float32)
    b2 = np.asarray(b2, dtype=np.float32)
    Wd = np.asarray(Wd, dtype=np.float32)
    bd = np.asarray(bd, dtype=np.float32)

    W1h, W1c, W1h0 = W1[:H], W1[H:2 * H], W1[2 * H:]
    w1hp = W1h - INV * W1c
    w1ci = INV * W1c
    wp0 = w1hp + W1h0
    m2 = W2 @ w1hp
    mc = W2 @ w1ci
    md = W2 @ Wd
    bb = b2 @ (W1h + W1c)
    # r_s = relu(P_s + b1 + s*bb)
    bsteps = np.stack([b1 + s * bb for s in range(STEPS)], axis=1)  # [H, S]
    bdp = bd + STEPS * (b2 @ Wd)

    def pack(w):  # [H, out] -> [P, KT, out] bf16
        return np.ascontiguousarray(
            w.reshape(KT, P, w.shape[1]).transpose(1, 0, 2).astype(BF))

    shared = {
        "emb": np.ascontiguousarray(emb.astype(BF)),
        "wp0": pack(wp0),
        "m2": pack(m2),
        "mc": pack(mc),
        "w1ci": pack(w1ci),
        "wd": pack(Wd),
        "md": pack(md),
        "bsteps": np.ascontiguousarray(
            bsteps.reshape(KT, P, STEPS).transpose(1, 0, 2)),
        "bdp": np.ascontiguousarray(bdp.reshape(A, 1)),
    }
    in_maps = []
    for c in range(NCORES):
        ids_local = np.asarray(
            agent_ids[c * G:(c + 1) * G], dtype=np.int32).reshape(T)
        ids_pt = np.ascontiguousarray(ids_local.reshape(T // P, P).T)
        in_maps.append({"ids_pt": ids_pt, **shared})
    return in_maps


def _run(in_maps, trace=False, tmpdir=None):
    from concourse.bass_utils import run_bass_kernel_spmd

    if "nc" not in _CACHE:
        _CACHE["nc"] = _build()
    nc = _CACHE["nc"]
    res = run_bass_kernel_spmd(
        nc, in_maps, core_ids=list(range(NCORES)), trace=trace, tmpdir=tmpdir)
    out = np.empty((B, M, A), dtype=np.float32)
    for c in range(NCORES):
        logT = res.results[c]["logT"]  # [A, T]
        out[c * G:(c + 1) * G] = logT.T.reshape(G, M, A)
    return out, res


def kernel(agent_ids, emb, W1, b1, W2, b2, Wd, bd):
    in_maps = _prep_inputs(agent_ids, emb, W1, b1, W2, b2, Wd, bd)
    out, _ = _run(in_maps, trace=False)
    return out


# revision 12
# speedup vs baseline: 1.6091x; 1.2118x over previous
"""CommNet forward on 8 TRN2 NeuronCores (Bass/Tile).

Model (per reference):
    h0 = emb[agent_ids]                      # (B, M, H)
    repeat 4x:
        c = (sum_m h - h) / (M-1)
        x = [h, c, h0]                       # (B, M, 3H)
        d = relu(x @ W1 + b1) @ W2 + b2
        h = h + d
    logits = h @ Wd + bd                     # (B, M, A)

Constants: B=1024, M=64, H=256, A=16, V=1000, 4 comm steps.

Sharding: data-parallel on B across 8 cores (128 groups / core); weights
replicated. Within a core every tensor is [hidden-on-partitions,
tokens-on-free] (tokens = group*64 + agent, T=8192 per core).

Pre-activation formulation (state = PRE in PSUM, never materialize h):
    P_0 = (W1h - inv*W1c + W1h0)^T h0 + (inv*W1c)^T segsum(h0)
    r_s = relu(P_s + b1 + s*bb)        bb = b2 @ (W1h + W1c)
    P_{s+1} = P_s + (W2 @ W1hp)^T r_s + bcast((W2 @ inv*W1c)^T segsum(r_s))
    logits = Wd^T h0 + (W2 @ Wd)^T (sum_s r_s) + (bd + 4 b2 @ Wd)
P accumulates IN PSUM (fp32) across steps via matmul start=False; groups of
64 tokens never cross a 512-token chunk, so the kernel is a per-chunk
pipeline (chunk pairs interleaved for engine overlap).  sum_s r_s
accumulates into the logits PSUM bank per step.  All matmul operands are
BF16 (separate fast-loadable weights; fp32/f32r pay a serial inline weight
load per matmul); PSUM accumulation stays fp32.
"""

import numpy as np
import ml_dtypes

BF = ml_dtypes.bfloat16

B, M, H, A, V = 1024, 64, 256, 16, 1000
STEPS = 4
NCORES = 8
G = B // NCORES          # groups per core = 128
T = G * M                # tokens per core = 8192
P = 128                  # partitions
KT = H // P              # K tiles per H = 2
CH = 512                 # tokens per chunk
NCH = T // CH            # chunks = 16
GPC = CH // M            # groups per chunk = 8
TPC = CH // P            # 128-token tiles per chunk = 4
INV = 1.0 / (M - 1)

_CACHE = {}


def _build():
    import concourse.bass as bass
    import concourse.tile as tile
    from concourse import bacc, mybir
    from concourse.masks import make_identity

    F32 = mybir.dt.float32
    BF16 = mybir.dt.bfloat16
    I32 = mybir.dt.int32

    nc = bacc.Bacc("TRN2", target_bir_lowering=False, debug=False,
                   num_devices=NCORES)

    ids_d = nc.dram_tensor("ids_pt", [P, T // P], I32, kind="ExternalInput").ap()
    emb_d = nc.dram_tensor("emb", [V, H], BF16, kind="ExternalInput").ap()
    wp0_d = nc.dram_tensor("wp0", [P, KT, H], BF16, kind="ExternalInput").ap()
    m2_d = nc.dram_tensor("m2", [P, KT, H], BF16, kind="ExternalInput").ap()
    mc_d = nc.dram_tensor("mc", [P, KT, H], BF16, kind="ExternalInput").ap()
    w1ci_d = nc.dram_tensor("w1ci", [P, KT, H], BF16, kind="ExternalInput").ap()
    wd_d = nc.dram_tensor("wd", [P, KT, A], BF16, kind="ExternalInput").ap()
    md_d = nc.dram_tensor("md", [P, KT, A], BF16, kind="ExternalInput").ap()
    bs_d = nc.dram_tensor("bsteps", [P, KT, STEPS], F32, kind="ExternalInput").ap()
    bd_d = nc.dram_tensor("bdp", [A, 1], F32, kind="ExternalInput").ap()
    logT_d = nc.dram_tensor("logT", [A, T], F32, kind="ExternalOutput").ap()

    with tile.TileContext(nc) as tc:
        with (
            tc.tile_pool(name="const", bufs=1) as const,
            tc.tile_pool(name="gat", bufs=8) as gat,
            tc.tile_pool(name="h0p", bufs=2) as h0p,
            tc.tile_pool(name="rp", bufs=2) as rp,
            tc.tile_pool(name="Rp", bufs=2) as Rpl,
            tc.tile_pool(name="swp", bufs=2) as swp,
            tc.tile_pool(name="lgt", bufs=2) as lgt,
            tc.tile_pool(name="prep", bufs=2, space="PSUM") as prep,
            tc.tile_pool(name="lgp", bufs=2, space="PSUM") as lgp,
            tc.tile_pool(name="scr", bufs=2, space="PSUM") as scr,
        ):
            # ---- constants / weights -------------------------------------
            ids = const.tile([P, T // P], I32)
            nc.sync.dma_start(out=ids[:], in_=ids_d[:])

            identf = const.tile([P, P], F32)
            make_identity(nc, identf[:])
            ident = const.tile([P, P], BF16)
            nc.vector.tensor_copy(ident[:], identf[:])

            warm = scr.tile([P, P], F32, space="PSUM", tag="scr", bufs=2,
                            name="warm")
            for _ in range(24):
                nc.tensor.matmul(warm[:], ident[:], ident[:],
                                 start=True, stop=True)

            wp0 = const.tile([P, KT, H], BF16, name="wp0")
            nc.sync.dma_start(out=wp0[:], in_=wp0_d[:])
            m2 = const.tile([P, KT, H], BF16, name="m2")
            nc.sync.dma_start(out=m2[:], in_=m2_d[:])
            mc = const.tile([P, KT, H], BF16, name="mc")
            nc.sync.dma_start(out=mc[:], in_=mc_d[:])
            w1ci = const.tile([P, KT, H], BF16, name="w1ci")
            nc.sync.dma_start(out=w1ci[:], in_=w1ci_d[:])
            wd = const.tile([P, KT, A], BF16, name="wd")
            nc.sync.dma_start(out=wd[:], in_=wd_d[:])
            md = const.tile([P, KT, A], BF16, name="md")
            nc.sync.dma_start(out=md[:], in_=md_d[:])
            bsteps = const.tile([P, KT, STEPS], F32)
            nc.sync.dma_start(out=bsteps[:], in_=bs_d[:])
            bd = const.tile([A, 1], F32)
            nc.sync.dma_start(out=bd[:], in_=bd_d[:])

            NPAIR = NCH // 2

            def gather_pair(p):
                """Issue the 8 indirect gathers for pair p; return gst tiles."""
                gsts = []
                for ci in range(2):
                    q = 2 * p + ci
                    for tl in range(TPC):
                        t = q * TPC + tl
                        gst = gat.tile([P, H], BF16, tag="gst",
                                       name=f"gst{q}_{tl}")
                        nc.gpsimd.indirect_dma_start(
                            out=gst[:],
                            out_offset=None,
                            in_=emb_d[:],
                            in_offset=bass.IndirectOffsetOnAxis(
                                ap=ids[:, t:t + 1], axis=0),
                        )
                        gsts.append(gst)
                return gsts

            def alloc_h0c():
                return {(ci, j): h0p.tile([P, CH], BF16, tag=f"h0c{ci}{j}",
                                          name="h0c")
                        for ci in range(2) for j in range(KT)}

            def transpose_closures(gsts, h0c):
                """One closure per (ci,tl,k): PE transpose via scr psum,
                then copy to h0c SBUF (vector/scalar alternating)."""
                def mk(ci, tl, k):
                    def run():
                        gst = gsts[ci * TPC + tl]
                        pt = scr.tile([P, P], BF16, space="PSUM",
                                      tag="scr", bufs=2, name="pt")
                        nc.tensor.transpose(
                            out=pt[:], in_=gst[:, k * P:(k + 1) * P],
                            identity=ident[:])
                        hdst = h0c[ci, k][:, tl * P:(tl + 1) * P]
                        if (tl + k) % 2 == 0:
                            nc.vector.tensor_copy(hdst, pt[:])
                        else:
                            nc.scalar.activation(
                                out=hdst, in_=pt[:],
                                func=mybir.ActivationFunctionType.Identity)
                    return run
                return [mk(ci, tl, k) for ci in range(2)
                        for tl in range(TPC) for k in range(KT)]

            def process_pair(p, h0c, fill):
                # ---- per-chunk state tiles -------------------------------
                pre = {}    # (ci, j) -> psum tile [P, CH] fp32
                for ci in range(2):
                    for j in range(KT):
                        pre[ci, j] = prep.tile([P, CH], F32, space="PSUM",
                                               tag=f"pre{j}", name="pre")

                def fill_some(n):
                    for _ in range(min(n, len(fill))):
                        fill.pop(0)()

                # ---- segsum(h0) for both chunks --------------------------
                R0 = Rpl.tile([P, KT, 2 * GPC], BF16, tag="R", name="R0")
                with nc.allow_low_precision(reason="segsum of 64 bf16 vals"):
                    for ci in range(2):
                        for k in range(KT):
                            nc.vector.tensor_reduce(
                                out=R0[:, k, ci * GPC:(ci + 1) * GPC],
                                in_=h0c[ci, k][:].rearrange(
                                    "p (g m) -> p g m", m=M),
                                axis=mybir.AxisListType.X,
                                op=mybir.AluOpType.add)

                def dsw_inject(Rt, wmat, start):
                    """psw = wmat^T @ Rt (both chunks), broadcast-inject into
                    pre[ci,j]."""
                    psw = scr.tile([P, KT * 2 * GPC], F32, space="PSUM",
                                   tag="scr", bufs=2, name="psw")
                    for j in range(KT):
                        for k in range(KT):
                            nc.tensor.matmul(
                                psw[:, j * 2 * GPC:(j + 1) * 2 * GPC],
                                wmat[:, k, j * P:(j + 1) * P],
                                Rt[:, k, :],
                                start=(k == 0), stop=(k == KT - 1))
                    swd = swp.tile([P, KT, 2 * GPC], BF16, tag="swd",
                                   name="swd")
                    nc.vector.tensor_copy(
                        swd[:], psw[:].rearrange("p (j g) -> p j g", j=KT))
                    for ci in range(2):
                        for j in range(KT):
                            nc.tensor.matmul(
                                pre[ci, j][:].rearrange(
                                    "p (g m) -> p g m", g=GPC),
                                ident[:],
                                swd[:, j, ci * GPC:(ci + 1) * GPC]
                                .to_broadcast([P, GPC, M]),
                                start=False, stop=True,
                                skip_group_check=not start)

                # ---- P_0 = wp0^T h0 (+ SW_0 inject closes the group) -----
                for ci in range(2):
                    for j in range(KT):
                        for k in range(KT):
                            nc.tensor.matmul(
                                pre[ci, j][:], wp0[:, k, j * P:(j + 1) * P],
                                h0c[ci, k][:],
                                start=(k == 0), stop=False)
                dsw_inject(R0, w1ci, start=True)

                # ---- logits psum: init with Wd^T h0 ----------------------
                lgps = {}
                for ci in range(2):
                    pl = lgp.tile([A, CH], F32, space="PSUM", tag="lg",
                                  name="pl")
                    lgps[ci] = pl
                    for k in range(KT):
                        nc.tensor.matmul(pl[:], wd[:, k, :], h0c[ci, k][:],
                                         start=(k == 0), stop=(k == KT - 1))

                # ---- comm steps ------------------------------------------
                for s in range(STEPS):
                    last = s == STEPS - 1
                    r = {}
                    for ci in range(2):
                        for j in range(KT):
                            rt = rp.tile([P, CH], BF16, tag=f"r{ci}{j}",
                                         name="r")
                            nc.scalar.activation(
                                out=rt[:], in_=pre[ci, j][:],
                                func=mybir.ActivationFunctionType.Relu,
                                bias=bsteps[:, j, s:s + 1])
                            r[ci, j] = rt
                    # fill the relu-wait PE bubble with next pair's work
                    fill_some(4)
                    # logits += md^T r_s   (cross-step psum accumulate)
                    for ci in range(2):
                        for k in range(KT):
                            nc.tensor.matmul(
                                lgps[ci][:], md[:, k, :], r[ci, k][:],
                                start=False, stop=False,
                                skip_group_check=True)
                    if last:
                        break
                    # segsum(r) for both chunks
                    Rt = Rpl.tile([P, KT, 2 * GPC], BF16, tag="R", name="Rt")
                    with nc.allow_low_precision(
                            reason="segsum of 64 bf16 vals"):
                        for ci in range(2):
                            for k in range(KT):
                                nc.vector.tensor_reduce(
                                    out=Rt[:, k, ci * GPC:(ci + 1) * GPC],
                                    in_=r[ci, k][:].rearrange(
                                        "p (g m) -> p g m", m=M),
                                    axis=mybir.AxisListType.X,
                                    op=mybir.AluOpType.add)
                    # pre += m2^T r   (cross-step accumulate, group ended)
                    for ci in range(2):
                        for j in range(KT):
                            for k in range(KT):
                                nc.tensor.matmul(
                                    pre[ci, j][:], m2[:, k, j * P:(j + 1) * P],
                                    r[ci, k][:],
                                    start=False, stop=False,
                                    skip_group_check=True)
                    # pre += bcast(mc^T segsum(r))
                    dsw_inject(Rt, mc, start=False)

                fill_some(len(fill))
                # ---- logits out ------------------------------------------
                for ci in range(2):
                    lg = lgt.tile([A, CH], F32, tag="lg")
                    nc.scalar.activation(
                        out=lg[:], in_=lgps[ci][:],
                        func=mybir.ActivationFunctionType.Identity,
                        bias=bd[:, 0:1])
                    q = 2 * p + ci
                    nc.sync.dma_start(
                        out=logT_d[:, q * CH:(q + 1) * CH], in_=lg[:])

            gsts = gather_pair(0)
            h0c_cur = alloc_h0c()
            for f in transpose_closures(gsts, h0c_cur):
                f()
            for p in range(NPAIR):
                if p + 1 < NPAIR:
                    nxt_gsts = gather_pair(p + 1)
                    h0c_nxt = alloc_h0c()
                    fill = transpose_closures(nxt_gsts, h0c_nxt)
                else:
                    h0c_nxt, fill = None, []
                process_pair(p, h0c_cur, fill)
                h0c_cur = h0c_nxt

    nc.compile()
    return nc


def _prep_inputs(agent_ids, emb, W1, b1, W2, b2, Wd, bd):
    agent_ids = np.asarray(agent_ids)
    emb = np.asarray(emb, dtype=np.float32)
    W1 = np.asarray(W1, dtype=np.float32)
    b1 = np.asarray(b1, dtype=np.float32)
    W2 = np.asarray(W2, dtype=np.float32)
    b2 = np.asarray(b2, dtype=np.float32)
    Wd = np.asarray(Wd, dtype=np.float32)
    bd = np.asarray(bd, dtype=np.float32)

    W1h, W1c, W1h0 = W1[:H], W1[H:2 * H], W1[2 * H:]
    w1hp = W1h - INV * W1c
    w1ci = INV * W1c
    wp0 = w1hp + W1h0
    m2 = W2 @ w1hp
    mc = W2 @ w1ci
    md = W2 @ Wd
    bb = b2 @ (W1h + W1c)
    # r_s = relu(P_s + b1 + s*bb)
    bsteps = np.stack([b1 + s * bb for s in range(STEPS)], axis=1)  # [H, S]
    bdp = bd + STEPS * (b2 @ Wd)

    def pack(w):  # [H, out] -> [P, KT, out] bf16
        return np.ascontiguousarray(
            w.reshape(KT, P, w.shape[1]).transpose(1, 0, 2).astype(BF))

    shared = {
        "emb": np.ascontiguousarray(emb.astype(BF)),
        "wp0": pack(wp0),
        "m2": pack(m2),
        "mc": pack(mc),
        "w1ci": pack(w1ci),
        "wd": pack(Wd),
        "md": pack(md),
        "bsteps": np.ascontiguousarray(
            bsteps.reshape(KT, P, STEPS).transpose(1, 0, 2)),
        "bdp": np.ascontiguousarray(bdp.reshape(A, 1)),
    }
    in_maps = []
    for c in range(NCORES):
        ids_local = np.asarray(
            agent_ids[c * G:(c + 1) * G], dtype=np.int32).reshape(T)
        ids_pt = np.ascontiguousarray(ids_local.reshape(T // P, P).T)
        in_maps.append({"ids_pt": ids_pt, **shared})
    return in_maps


def _run(in_maps, trace=False, tmpdir=None):
    from concourse.bass_utils import run_bass_kernel_spmd

    if "nc" not in _CACHE:
        _CACHE["nc"] = _build()
    nc = _CACHE["nc"]
    res = run_bass_kernel_spmd(
        nc, in_maps, core_ids=list(range(NCORES)), trace=trace, tmpdir=tmpdir)
    out = np.empty((B, M, A), dtype=np.float32)
    for c in range(NCORES):
        logT = res.results[c]["logT"]  # [A, T]
        out[c * G:(c + 1) * G] = logT.T.reshape(G, M, A)
    return out, res


def kernel(agent_ids, emb, W1, b1, W2, b2, Wd, bd):
    in_maps = _prep_inputs(agent_ids, emb, W1, b1, W2, b2, Wd, bd)
    out, _ = _run(in_maps, trace=False)
    return out
